# revision 17
# baseline (speedup 1.0000x reference)
"""Trainium2 Bass kernel for nn_Attention_42279658062045 (gnn_message_passing).

Computes, for each of B=200000 nodes:
    simi   = exp(-source_distance^2 / 2)                  [B, K]
    weight = softmax(simi @ kernel + bias, axis=-1)       [B, K]
    mean   = einsum('bk,bkd->bd', weight, context)        [B, D]

Sharding: pure data parallel over the node axis B across 8 NeuronCores;
kernel/bias replicated; no cross-device communication.

Per-core dataflow (B_LOCAL = 25000 rows, tiles of 128 rows, f32 throughout).
Rows are mapped to (partition, tile) as row = p*195 + t ("(p n)" layout), so
the source_distance load and the output store are fully contiguous per
partition (large DMA descriptors); context chunk descriptors are
30.7 KB/partition.

Work split per 128-row tile (K=30 weighted slabs of D=64). The softmax 1/Z
is applied to the 30 WEIGHTS (not the 64 outputs), so the k-reduction writes
the final output directly:
  - PE: transpose of simi tiles; logits = simi @ kernel + bias via two
    accumulating matmuls (ones-row stationary adds the bias).
  - ACT: batched square+exp of the distances, PSUM->SBUF copy of simi^T,
    exp(logits) with accum_out giving Z, the weight normalization
    wn = ew * (1/Z), and the weighted product for ACT_SLABS k-slabs.
  - GPSIMD: the weighted product for GP_SLABS k-slabs.
  - DVE: 1/Z reciprocal, weighted product for DVE_SLABS, and the full-30
    k-reduction over the interleaved product.

Engine-contention notes (HW-measured): a fp32 DVE tensor_tensor with two
SBUF input streams rides the SBUF read port shared with GPSIMD and runs
2-4x slower under overlap, so the DVE product reads its broadcast weight
operand from PSUM instead (dedicated PSUM read port; ACT writes a second
copy of the normalized weights there). fp32 tensor_reduce and reciprocal
are single-stream (1x perf mode) and are immune. No DVE op here is a
2-port perf-mode op (fp32 single-src copy/scalar), which would take the
shared port pair as an exclusive lock and block GPSIMD entirely.

The product tensor uses an interleaved layout [d_hi(32), k(30), d_lo(2)]
(flat addr = d_hi*60 + k*2 + d_lo) so the k-reduction reads at 8-byte
stride (2 hits per 16-byte SBUF line = full rate).
"""

import numpy as np

N_CORES = 8
B, K, D = 200000, 30, 64
B_LOCAL = B // N_CORES  # 25000
P = 128
CT = 3          # tiles per context DMA chunk
IL = 2          # product interleave: [d_hi(32), k(30), d_lo(IL)]
DH = D // IL    # 32
DVE_SLABS = 6   # k in [0, DVE_SLABS) multiplied on DVE
GP_SLABS = 21   # k in [DVE_SLABS, GP_END) multiplied on GPSIMD
GP_END = DVE_SLABS + GP_SLABS
ACT_SLABS = K - GP_END  # remaining slabs multiplied on ACT

_CACHE = {}


def _build():
    import concourse.bacc as bacc
    import concourse.tile as tile
    from concourse import mybir
    from concourse.masks import make_identity

    fp32 = mybir.dt.float32
    AF = mybir.ActivationFunctionType
    nc = bacc.Bacc("TRN2", target_bir_lowering=False, debug=False,
                   num_devices=N_CORES)

    dist = nc.dram_tensor("source_distance", [B_LOCAL, K], fp32,
                          kind="ExternalInput").ap()
    ctx_d = nc.dram_tensor("context", [B_LOCAL, K, D], fp32,
                           kind="ExternalInput").ap()
    kern = nc.dram_tensor("kernel", [K, K], fp32, kind="ExternalInput").ap()
    bias = nc.dram_tensor("bias", [K], fp32, kind="ExternalInput").ap()
    out = nc.dram_tensor("out", [B_LOCAL, D], fp32, kind="ExternalOutput").ap()

    n_full = B_LOCAL // P          # 195 full tiles
    rem = B_LOCAL - n_full * P     # 40 leftover rows

    # row(p, t) = p*n_full + t: per-partition-contiguous HBM runs.
    dist_v = dist[:n_full * P, :].rearrange("(p n) k -> p n k", p=P)
    ctx_v = ctx_d[:n_full * P].rearrange("(p n) k d -> p n (k d)", p=P)
    out_v = out[:n_full * P, :].rearrange("(p n) d -> p n d", p=P)

    with tile.TileContext(nc) as tc:
        from contextlib import ExitStack
        with ExitStack() as st:
            consts = st.enter_context(tc.tile_pool(name="consts", bufs=1))
            big = st.enter_context(tc.tile_pool(name="big", bufs=1))
            ctxp = st.enter_context(tc.tile_pool(name="ctx", bufs=3))
            prodp = st.enter_context(tc.tile_pool(name="prod", bufs=4))
            small = st.enter_context(tc.tile_pool(name="small", bufs=5))
            remp = st.enter_context(tc.tile_pool(name="rem", bufs=1))
            rzp = st.enter_context(tc.tile_pool(name="rz", bufs=10))
            psum_t = st.enter_context(
                tc.tile_pool(name="psumT", bufs=2, space="PSUM"))
            psum_l = st.enter_context(
                tc.tile_pool(name="psumL", bufs=2, space="PSUM"))
            psum_w = st.enter_context(
                tc.tile_pool(name="psumW", bufs=4, space="PSUM"))

            ident = consts.tile([P, P], fp32)
            make_identity(nc, ident)
            kern_s = consts.tile([K, K], fp32)
            nc.sync.dma_start(out=kern_s, in_=kern)
            bias_s = consts.tile([1, K], fp32)
            nc.sync.dma_start(out=bias_s, in_=bias.unsqueeze(0))
            ones_s = consts.tile([1, P], fp32)
            nc.vector.memset(ones_s, 1.0)

            # All distances for the full tiles; squared+exp'd in place.
            # Loaded lazily in 12-tile slices (a big monolithic square/exp
            # would monopolize ACT for ~3us and bubble the pipeline).
            simi_all = big.tile([P, n_full, K], fp32)
            SL = 12

            def load_simi_slice(r0):
                r1 = min(r0 + SL, n_full)
                nc.sync.dma_start(out=simi_all[:, r0:r1, :],
                                  in_=dist_v[:, r0:r1, :])
                nc.scalar.activation(out=simi_all[:, r0:r1, :],
                                     in_=simi_all[:, r0:r1, :], func=AF.Square)
                nc.scalar.activation(out=simi_all[:, r0:r1, :],
                                     in_=simi_all[:, r0:r1, :], func=AF.Exp,
                                     scale=-0.5)

            # Staged output for the full tiles.
            mean_all = big.tile([P, n_full, D], fp32)

            def logits_exp(simi_ap, expw_ap, zs_ap, rows):
                """simi [rows, K] -> expw = exp(simi @ kernel + bias),
                zs = sum(expw) per row."""
                simiT_p = psum_t.tile([K, P], fp32, tag="simiT_p")
                nc.tensor.transpose(out=simiT_p[:, :rows], in_=simi_ap,
                                    identity=ident[:rows, :rows])
                simiT_s = small.tile([K, P], fp32, tag="simiT_s")
                nc.scalar.copy(out=simiT_s[:, :rows], in_=simiT_p[:, :rows])

                logits_p = psum_l.tile([P, K], fp32, tag="logits_p")
                nc.tensor.matmul(out=logits_p[:rows, :],
                                 lhsT=simiT_s[:, :rows], rhs=kern_s,
                                 start=True, stop=False)
                nc.tensor.matmul(out=logits_p[:rows, :],
                                 lhsT=ones_s[:, :rows], rhs=bias_s,
                                 start=False, stop=True)

                nc.scalar.activation(out=expw_ap, in_=logits_p[:rows, :],
                                     func=AF.Exp, accum_out=zs_ap)

            # Remainder rows (partial tile): emitted first so its serial
            # chain overlaps the main pipeline.
            if rem:
                simi_r = remp.tile([P, K], fp32, tag="simi_r")
                nc.sync.dma_start(out=simi_r[:rem, :], in_=dist[n_full * P:, :])
                nc.scalar.activation(out=simi_r[:rem, :], in_=simi_r[:rem, :],
                                     func=AF.Square)
                nc.scalar.activation(out=simi_r[:rem, :], in_=simi_r[:rem, :],
                                     func=AF.Exp, scale=-0.5)
                ctx_r = ctxp.tile([P, K * D], fp32, tag="ctx")
                nc.sync.dma_start(
                    out=ctx_r[:rem, :],
                    in_=ctx_d[n_full * P:].rearrange("b k d -> b (k d)"))
                expw_r = remp.tile([P, K], fp32, tag="expw_r")
                zs_r = rzp.tile([P, 1], fp32, tag="zs_r")
                logits_exp(simi_r[:rem, :], expw_r[:rem, :], zs_r[:rem, :], rem)
                rz_r = rzp.tile([P, 1], fp32, tag="rz_r")
                nc.vector.reciprocal(out=rz_r[:rem, :], in_=zs_r[:rem, :])
                wn_r = remp.tile([P, K], fp32, tag="wn_r")
                nc.scalar.mul(out=wn_r[:rem, :], in_=expw_r[:rem, :],
                              mul=rz_r[:rem, :])
                pp_r = prodp.tile([P, 2, DH, K, IL], fp32, tag="prod")
                prod_r = pp_r[:, 0]
                ctx3r = ctx_r[:rem, :].rearrange("p (k d) -> p k d", k=K)
                nc.vector.tensor_mul(
                    out=prod_r.rearrange("p h k l -> p k h l")[:rem],
                    in0=ctx3r,
                    in1=wn_r[:rem, :].unsqueeze(2).broadcast_to([rem, K, D]))
                mean_r = remp.tile([P, D], fp32, tag="mean_r")
                nc.vector.reduce_sum(
                    out=mean_r[:rem, :].rearrange("p (h l) -> p h l", l=IL),
                    in_=prod_r.rearrange("p h k l -> p h l k")[:rem],
                    axis=mybir.AxisListType.X)
                nc.sync.dma_start(out=out[n_full * P:, :], in_=mean_r[:rem, :])

            # Full tiles: context DMA in CT-tile chunks; per-tile products
            # (interleaved) split DVE/GPSIMD/ACT, full-30 DVE reduce writing
            # the output tile directly.
            #
            # Software pipelining: the logits/exp chain (phase A) is emitted
            # one tile ahead of the product/reduce work (phase B). Products
            # use UNNORMALIZED exp weights, so they depend only on the exp;
            # the 1/Z is applied to the reduced pair outputs by GPSIMD, one
            # pair late, and the pair reduce itself is emitted one pair
            # late, so each pair's product ops precede the previous pair's
            # 4.1us reduce in the DVE queue and GPSIMD never starves.
            ews = {}
            zpairs = {}
            rzpairs = {}
            prodpairs = {}

            def phase_a(t):
                if t == 0:
                    load_simi_slice(0)
                elif (t + 4) % SL == 0 and t + 4 < n_full:
                    load_simi_slice(t + 4)
                ew = small.tile([P, K], fp32, tag="ew")
                if t % 2 == 0:
                    zpair = rzp.tile([P, 2], fp32, tag="zpair")
                    zpairs[t] = zpair
                zp = zpairs[t - t % 2]
                logits_exp(simi_all[:, t, :], ew, zp[:, t % 2:t % 2 + 1], P)
                ews[t] = ew
                if t % 2 == 1:
                    # one tiny immune DVE op per pair; consumed much later
                    # by the GPSIMD output normalization.
                    rzpair = rzp.tile([P, 2], fp32, tag="rzpair")
                    nc.vector.reciprocal(out=rzpair, in_=zp)
                    rzpairs[t - 1] = rzpair
                    del zpairs[t - 1]

            def emit_reduce(j0):
                """Reduce pair (j0, j0+1) -> mean_all; frees the prodpair."""
                pp = prodpairs.pop(j0)
                nc.vector.reduce_sum(
                    out=mean_all[:, j0:j0 + 2, :].rearrange(
                        "p j (h l) -> p j h l", l=IL),
                    in_=pp.rearrange("p j h k l -> p j h l k"),
                    axis=mybir.AxisListType.X)

            def emit_norm(j0, w=2):
                """ACT: mean_all pair (j0, j0+1) *= 1/Z (in place)."""
                rzp_t = rzpairs.pop(j0)
                for j in range(w):
                    nc.scalar.mul(out=mean_all[:, j0 + j, :],
                                  in_=mean_all[:, j0 + j, :],
                                  mul=rzp_t[:, j:j + 1])

            ctx_tile = None
            phase_a(0)
            for t in range(n_full):
                cc, lane = divmod(t, CT)
                if lane == 0:
                    ctx_tile = ctxp.tile([P, CT, K * D], fp32, tag="ctx")
                    lo = cc * CT
                    cn = min(CT, n_full - lo)
                    nc.sync.dma_start(out=ctx_tile[:, :cn, :],
                                      in_=ctx_v[:, lo:lo + cn, :])
                ctx3 = ctx_tile[:, lane, :].rearrange("p (k d) -> p k d", k=K)

                if t + 1 < n_full:
                    phase_a(t + 1)

                ew = ews.pop(t)
                # PSUM copy of the DVE slabs' weights (dedicated DVE read
                # port; avoids the GPSIMD-shared SBUF port).
                ew_p = psum_w.tile([P, DVE_SLABS], fp32, tag="ew_p")
                nc.scalar.copy(out=ew_p, in_=ew[:, :DVE_SLABS])

                if t % 2 == 0:
                    prodpair_new = prodp.tile([P, 2, DH, K, IL], fp32,
                                              tag="prod")
                    prodpairs[t] = prodpair_new
                prodpair = prodpairs[t - t % 2]
                prod = prodpair[:, t % 2]
                prod_k = prod.rearrange("p h k l -> p k h l")
                # GPSIMD: product for slabs [DVE_SLABS, GP_END)
                nc.gpsimd.tensor_mul(
                    out=prod_k[:, DVE_SLABS:GP_END],
                    in0=ctx3[:, DVE_SLABS:GP_END, :],
                    in1=ew[:, DVE_SLABS:GP_END].unsqueeze(2).broadcast_to(
                        [P, GP_SLABS, D]))
                # DVE: product for slabs [0, DVE_SLABS), interleaved out
                nc.vector.tensor_mul(
                    out=prod_k[:, :DVE_SLABS],
                    in0=ctx3[:, :DVE_SLABS, :],
                    in1=ew_p.unsqueeze(2).broadcast_to([P, DVE_SLABS, D]))
                # ACT: product for the last ACT_SLABS slabs
                for k in range(GP_END, K):
                    nc.scalar.mul(out=prod[:, :, k, :],
                                  in_=ctx3[:, k, :], mul=ew[:, k:k + 1])

                # DVE: full-30 reduce at 8-byte stride over the PREVIOUS
                # pair (one-pair lag keeps this pair's product ops ahead
                # of it in the DVE queue).
                if t % 2 == 1 and t >= 3:
                    emit_reduce(t - 3)
                # ACT: 1/Z normalization of the pair reduced at t-3
                # (two-pair lag: the reduce is certainly done, so the norm
                # never head-of-line-blocks the ACT queue).
                if t % 2 == 0 and t >= 6:
                    emit_norm(t - 6)
                # stream the staged output in 8-tile slices of normalized
                # tiles (highest normalized tile after emit_norm: t-5).
                if t % 2 == 0 and t >= 6 and (t - 4) % 8 == 0:
                    nc.sync.dma_start(out=out_v[:, t - 12:t - 4, :],
                                      in_=mean_all[:, t - 12:t - 4, :])

            # drain: tiles 192, 193 (pair) and 194 (solo)
            emit_reduce(192)
            pp_last = prodpairs.pop(194)
            nc.vector.reduce_sum(
                out=mean_all[:, 194, :].rearrange("p (h l) -> p h l", l=IL),
                in_=pp_last[:, 0].rearrange("p h k l -> p h l k"),
                axis=mybir.AxisListType.X)
            emit_norm(190)
            emit_norm(192)
            rz194 = rzp.tile([P, 2], fp32, tag="rzpair")
            nc.vector.reciprocal(out=rz194[:, 0:1],
                                 in_=zpairs.pop(194)[:, 0:1])
            rzpairs[194] = rz194
            emit_norm(194, w=1)
            nc.sync.dma_start(out=out_v[:, 184:, :],
                              in_=mean_all[:, 184:, :])

    nc.compile()
    return nc


def _get_nc():
    if "nc" not in _CACHE:
        _CACHE["nc"] = _build()
    return _CACHE["nc"]


def kernel(source_distance, context, kernel, bias, _trace=False, _tmpdir=None):
    from concourse.bass_utils import run_bass_kernel_spmd

    nc = _get_nc()

    source_distance = np.ascontiguousarray(source_distance, dtype=np.float32)
    context = np.ascontiguousarray(context, dtype=np.float32)
    kernel = np.ascontiguousarray(kernel, dtype=np.float32)
    bias = np.ascontiguousarray(bias, dtype=np.float32)

    in_maps = []
    for i in range(N_CORES):
        lo, hi = i * B_LOCAL, (i + 1) * B_LOCAL
        in_maps.append({
            "source_distance": source_distance[lo:hi],
            "context": context[lo:hi],
            "kernel": kernel,
            "bias": bias,
        })

    res = run_bass_kernel_spmd(nc, in_maps, list(range(N_CORES)),
                               trace=_trace, tmpdir=_tmpdir)
    out = np.concatenate([res.results[i]["out"] for i in range(N_CORES)], axis=0)
    if _trace:
        _CACHE["last_results"] = res
    return out


# revision 18
# speedup vs baseline: 1.0523x; 1.0523x over previous
"""Trainium2 Bass kernel for nn_Attention_42279658062045 (gnn_message_passing).

Computes, for each of B=200000 nodes:
    simi   = exp(-source_distance^2 / 2)                  [B, K]
    weight = softmax(simi @ kernel + bias, axis=-1)       [B, K]
    mean   = einsum('bk,bkd->bd', weight, context)        [B, D]

Sharding: pure data parallel over the node axis B across 8 NeuronCores;
kernel/bias replicated; no cross-device communication.

Per-core dataflow (B_LOCAL = 25000 rows, tiles of 128 rows, f32 throughout).
Rows are mapped to (partition, tile) as row = p*195 + t ("(p n)" layout), so
the source_distance load and the output store are fully contiguous per
partition (large DMA descriptors); context chunk descriptors are
30.7 KB/partition.

Work split per 128-row tile (K=30 weighted slabs of D=64). The softmax 1/Z
is applied to the 30 WEIGHTS (not the 64 outputs), so the k-reduction writes
the final output directly:
  - PE: transpose of simi tiles; logits = simi @ kernel + bias via two
    accumulating matmuls (ones-row stationary adds the bias).
  - ACT: batched square+exp of the distances, PSUM->SBUF copy of simi^T,
    exp(logits) with accum_out giving Z, the weight normalization
    wn = ew * (1/Z), and the weighted product for ACT_SLABS k-slabs.
  - GPSIMD: the weighted product for GP_SLABS k-slabs.
  - DVE: 1/Z reciprocal, weighted product for DVE_SLABS, and the full-30
    k-reduction over the interleaved product.

Engine-contention notes (HW-measured): a fp32 DVE tensor_tensor with two
SBUF input streams rides the SBUF read port shared with GPSIMD and runs
2-4x slower under overlap, so the DVE product reads its broadcast weight
operand from PSUM instead (dedicated PSUM read port; ACT writes a second
copy of the normalized weights there). fp32 tensor_reduce and reciprocal
are single-stream (1x perf mode) and are immune. No DVE op here is a
2-port perf-mode op (fp32 single-src copy/scalar), which would take the
shared port pair as an exclusive lock and block GPSIMD entirely.

The product tensor uses an interleaved layout [d_hi(32), k(30), d_lo(2)]
(flat addr = d_hi*60 + k*2 + d_lo) so the k-reduction reads at 8-byte
stride (2 hits per 16-byte SBUF line = full rate).
"""

import numpy as np

N_CORES = 8
B, K, D = 200000, 30, 64
B_LOCAL = B // N_CORES  # 25000
P = 128
CT = 3          # tiles per context DMA chunk
IL = 2          # product interleave: [d_hi(32), k(30), d_lo(IL)]
DH = D // IL    # 32
DVE_SLABS = 6   # k in [0, DVE_SLABS) multiplied on DVE
GP_SLABS = 21   # k in [DVE_SLABS, GP_END) multiplied on GPSIMD
GP_END = DVE_SLABS + GP_SLABS
ACT_SLABS = K - GP_END  # remaining slabs multiplied on ACT

_CACHE = {}


def _build():
    import concourse.bacc as bacc
    import concourse.tile as tile
    from concourse import mybir
    from concourse.masks import make_identity

    fp32 = mybir.dt.float32
    AF = mybir.ActivationFunctionType
    nc = bacc.Bacc("TRN2", target_bir_lowering=False, debug=False,
                   num_devices=N_CORES)

    dist = nc.dram_tensor("source_distance", [B_LOCAL, K], fp32,
                          kind="ExternalInput").ap()
    ctx_d = nc.dram_tensor("context", [B_LOCAL, K, D], fp32,
                           kind="ExternalInput").ap()
    kern = nc.dram_tensor("kernel", [K, K], fp32, kind="ExternalInput").ap()
    bias = nc.dram_tensor("bias", [K], fp32, kind="ExternalInput").ap()
    out = nc.dram_tensor("out", [B_LOCAL, D], fp32, kind="ExternalOutput").ap()

    n_full = B_LOCAL // P          # 195 full tiles
    rem = B_LOCAL - n_full * P     # 40 leftover rows

    # row(p, t) = p*n_full + t: per-partition-contiguous HBM runs.
    dist_v = dist[:n_full * P, :].rearrange("(p n) k -> p n k", p=P)
    ctx_v = ctx_d[:n_full * P].rearrange("(p n) k d -> p n (k d)", p=P)
    out_v = out[:n_full * P, :].rearrange("(p n) d -> p n d", p=P)

    with tile.TileContext(nc) as tc:
        from contextlib import ExitStack
        with ExitStack() as st:
            consts = st.enter_context(tc.tile_pool(name="consts", bufs=1))
            big = st.enter_context(tc.tile_pool(name="big", bufs=1))
            ctxp = st.enter_context(tc.tile_pool(name="ctx", bufs=3))
            prodp = st.enter_context(tc.tile_pool(name="prod", bufs=4))
            small = st.enter_context(tc.tile_pool(name="small", bufs=5))
            remp = st.enter_context(tc.tile_pool(name="rem", bufs=1))
            rzp = st.enter_context(tc.tile_pool(name="rz", bufs=10))
            psum_t = st.enter_context(
                tc.tile_pool(name="psumT", bufs=2, space="PSUM"))
            psum_l = st.enter_context(
                tc.tile_pool(name="psumL", bufs=2, space="PSUM"))
            psum_w = st.enter_context(
                tc.tile_pool(name="psumW", bufs=4, space="PSUM"))

            ident = consts.tile([P, P], fp32)
            make_identity(nc, ident)
            kern_s = consts.tile([K, K], fp32)
            nc.sync.dma_start(out=kern_s, in_=kern)
            bias_s = consts.tile([1, K], fp32)
            nc.sync.dma_start(out=bias_s, in_=bias.unsqueeze(0))
            ones_s = consts.tile([1, P], fp32)
            nc.vector.memset(ones_s, 1.0)

            # All distances for the full tiles; squared+exp'd in place.
            # The DMA comes in 4 big quarters (few, large transfers), but
            # the square/exp runs in 12-tile ACT slices (a monolithic
            # square/exp would monopolize ACT for ~3us and bubble the
            # pipeline).
            simi_all = big.tile([P, n_full, K], fp32)
            SL = 12
            simi_q = (n_full + 3) // 4

            def load_dist_quarter(r0):
                r1 = min(r0 + simi_q, n_full)
                nc.sync.dma_start(out=simi_all[:, r0:r1, :],
                                  in_=dist_v[:, r0:r1, :])

            def simi_prep_slice(r0):
                r1 = min(r0 + SL, n_full)
                nc.scalar.activation(out=simi_all[:, r0:r1, :],
                                     in_=simi_all[:, r0:r1, :], func=AF.Square)
                nc.scalar.activation(out=simi_all[:, r0:r1, :],
                                     in_=simi_all[:, r0:r1, :], func=AF.Exp,
                                     scale=-0.5)

            # Staged output for the full tiles.
            mean_all = big.tile([P, n_full, D], fp32)

            def logits_exp(simi_ap, expw_ap, zs_ap, rows):
                """simi [rows, K] -> expw = exp(simi @ kernel + bias),
                zs = sum(expw) per row."""
                simiT_p = psum_t.tile([K, P], fp32, tag="simiT_p")
                nc.tensor.transpose(out=simiT_p[:, :rows], in_=simi_ap,
                                    identity=ident[:rows, :rows])
                simiT_s = small.tile([K, P], fp32, tag="simiT_s")
                nc.scalar.copy(out=simiT_s[:, :rows], in_=simiT_p[:, :rows])

                logits_p = psum_l.tile([P, K], fp32, tag="logits_p")
                nc.tensor.matmul(out=logits_p[:rows, :],
                                 lhsT=simiT_s[:, :rows], rhs=kern_s,
                                 start=True, stop=False)
                nc.tensor.matmul(out=logits_p[:rows, :],
                                 lhsT=ones_s[:, :rows], rhs=bias_s,
                                 start=False, stop=True)

                nc.scalar.activation(out=expw_ap, in_=logits_p[:rows, :],
                                     func=AF.Exp, accum_out=zs_ap)

            # Remainder rows (partial tile): emitted first so its serial
            # chain overlaps the main pipeline.
            if rem:
                simi_r = remp.tile([P, K], fp32, tag="simi_r")
                nc.sync.dma_start(out=simi_r[:rem, :], in_=dist[n_full * P:, :])
                nc.scalar.activation(out=simi_r[:rem, :], in_=simi_r[:rem, :],
                                     func=AF.Square)
                nc.scalar.activation(out=simi_r[:rem, :], in_=simi_r[:rem, :],
                                     func=AF.Exp, scale=-0.5)
                ctx_r = ctxp.tile([P, K * D], fp32, tag="ctx")
                nc.sync.dma_start(
                    out=ctx_r[:rem, :],
                    in_=ctx_d[n_full * P:].rearrange("b k d -> b (k d)"))
                expw_r = remp.tile([P, K], fp32, tag="expw_r")
                zs_r = rzp.tile([P, 1], fp32, tag="zs_r")
                logits_exp(simi_r[:rem, :], expw_r[:rem, :], zs_r[:rem, :], rem)
                rz_r = rzp.tile([P, 1], fp32, tag="rz_r")
                nc.vector.reciprocal(out=rz_r[:rem, :], in_=zs_r[:rem, :])
                wn_r = remp.tile([P, K], fp32, tag="wn_r")
                nc.scalar.mul(out=wn_r[:rem, :], in_=expw_r[:rem, :],
                              mul=rz_r[:rem, :])
                pp_r = prodp.tile([P, 2, DH, K, IL], fp32, tag="prod")
                prod_r = pp_r[:, 0]
                ctx3r = ctx_r[:rem, :].rearrange("p (k d) -> p k d", k=K)
                nc.vector.tensor_mul(
                    out=prod_r.rearrange("p h k l -> p k h l")[:rem],
                    in0=ctx3r,
                    in1=wn_r[:rem, :].unsqueeze(2).broadcast_to([rem, K, D]))
                mean_r = remp.tile([P, D], fp32, tag="mean_r")
                nc.vector.reduce_sum(
                    out=mean_r[:rem, :].rearrange("p (h l) -> p h l", l=IL),
                    in_=prod_r.rearrange("p h k l -> p h l k")[:rem],
                    axis=mybir.AxisListType.X)
                nc.sync.dma_start(out=out[n_full * P:, :], in_=mean_r[:rem, :])

            # Full tiles: context DMA in CT-tile chunks; per-tile products
            # (interleaved) split DVE/GPSIMD/ACT, full-30 DVE reduce writing
            # the output tile directly.
            #
            # Software pipelining: the logits/exp chain (phase A) is emitted
            # one tile ahead of the product/reduce work (phase B). Products
            # use UNNORMALIZED exp weights, so they depend only on the exp;
            # the 1/Z is applied to the reduced pair outputs by GPSIMD, one
            # pair late, and the pair reduce itself is emitted one pair
            # late, so each pair's product ops precede the previous pair's
            # 4.1us reduce in the DVE queue and GPSIMD never starves.
            ews = {}
            zpairs = {}
            rzpairs = {}
            prodpairs = {}

            def phase_a(t):
                if t == 0:
                    load_dist_quarter(0)
                    simi_prep_slice(0)
                elif t in (37, 86, 135):
                    load_dist_quarter({37: 1, 86: 2, 135: 3}[t] * simi_q)
                if t > 0 and (t + 4) % SL == 0 and t + 4 < n_full:
                    simi_prep_slice(t + 4)
                ew = small.tile([P, K], fp32, tag="ew")
                if t % 2 == 0:
                    zpair = rzp.tile([P, 2], fp32, tag="zpair")
                    zpairs[t] = zpair
                zp = zpairs[t - t % 2]
                logits_exp(simi_all[:, t, :], ew, zp[:, t % 2:t % 2 + 1], P)
                ews[t] = ew
                if t % 2 == 1:
                    # one tiny immune DVE op per pair; consumed much later
                    # by the GPSIMD output normalization.
                    rzpair = rzp.tile([P, 2], fp32, tag="rzpair")
                    nc.vector.reciprocal(out=rzpair, in_=zp)
                    rzpairs[t - 1] = rzpair
                    del zpairs[t - 1]

            def emit_reduce(j0):
                """Reduce pair (j0, j0+1) -> mean_all; frees the prodpair."""
                pp = prodpairs.pop(j0)
                nc.vector.reduce_sum(
                    out=mean_all[:, j0:j0 + 2, :].rearrange(
                        "p j (h l) -> p j h l", l=IL),
                    in_=pp.rearrange("p j h k l -> p j h l k"),
                    axis=mybir.AxisListType.X)

            def emit_norm(j0, w=2):
                """ACT: mean_all pair (j0, j0+1) *= 1/Z (in place)."""
                rzp_t = rzpairs.pop(j0)
                for j in range(w):
                    nc.scalar.mul(out=mean_all[:, j0 + j, :],
                                  in_=mean_all[:, j0 + j, :],
                                  mul=rzp_t[:, j:j + 1])

            ctx_tile = None
            phase_a(0)
            for t in range(n_full):
                cc, lane = divmod(t, CT)
                if lane == 0:
                    ctx_tile = ctxp.tile([P, CT, K * D], fp32, tag="ctx")
                    lo = cc * CT
                    cn = min(CT, n_full - lo)
                    nc.sync.dma_start(out=ctx_tile[:, :cn, :],
                                      in_=ctx_v[:, lo:lo + cn, :])
                ctx3 = ctx_tile[:, lane, :].rearrange("p (k d) -> p k d", k=K)

                if t + 1 < n_full:
                    phase_a(t + 1)

                ew = ews.pop(t)
                # PSUM copy of the DVE slabs' weights (dedicated DVE read
                # port; avoids the GPSIMD-shared SBUF port).
                ew_p = psum_w.tile([P, DVE_SLABS], fp32, tag="ew_p")
                nc.scalar.copy(out=ew_p, in_=ew[:, :DVE_SLABS])

                if t % 2 == 0:
                    prodpair_new = prodp.tile([P, 2, DH, K, IL], fp32,
                                              tag="prod")
                    prodpairs[t] = prodpair_new
                prodpair = prodpairs[t - t % 2]
                prod = prodpair[:, t % 2]
                prod_k = prod.rearrange("p h k l -> p k h l")
                # GPSIMD: product for slabs [DVE_SLABS, GP_END)
                nc.gpsimd.tensor_mul(
                    out=prod_k[:, DVE_SLABS:GP_END],
                    in0=ctx3[:, DVE_SLABS:GP_END, :],
                    in1=ew[:, DVE_SLABS:GP_END].unsqueeze(2).broadcast_to(
                        [P, GP_SLABS, D]))
                # DVE: product for slabs [0, DVE_SLABS), interleaved out
                nc.vector.tensor_mul(
                    out=prod_k[:, :DVE_SLABS],
                    in0=ctx3[:, :DVE_SLABS, :],
                    in1=ew_p.unsqueeze(2).broadcast_to([P, DVE_SLABS, D]))
                # ACT: product for the last ACT_SLABS slabs
                for k in range(GP_END, K):
                    nc.scalar.mul(out=prod[:, :, k, :],
                                  in_=ctx3[:, k, :], mul=ew[:, k:k + 1])

                # DVE: full-30 reduce at 8-byte stride over the PREVIOUS
                # pair (one-pair lag keeps this pair's product ops ahead
                # of it in the DVE queue).
                if t % 2 == 1 and t >= 3:
                    emit_reduce(t - 3)
                # ACT: 1/Z normalization of the pair reduced at t-3
                # (two-pair lag: the reduce is certainly done, so the norm
                # never head-of-line-blocks the ACT queue).
                if t % 2 == 0 and t >= 6:
                    emit_norm(t - 6)
                # stream the staged output in 8-tile slices of normalized
                # tiles (highest normalized tile after emit_norm: t-5).
                if t % 2 == 0 and t >= 6 and (t - 4) % 8 == 0:
                    nc.sync.dma_start(out=out_v[:, t - 12:t - 4, :],
                                      in_=mean_all[:, t - 12:t - 4, :])

            # drain: tiles 192, 193 (pair) and 194 (solo)
            emit_reduce(192)
            pp_last = prodpairs.pop(194)
            nc.vector.reduce_sum(
                out=mean_all[:, 194, :].rearrange("p (h l) -> p h l", l=IL),
                in_=pp_last[:, 0].rearrange("p h k l -> p h l k"),
                axis=mybir.AxisListType.X)
            emit_norm(190)
            emit_norm(192)
            rz194 = rzp.tile([P, 2], fp32, tag="rzpair")
            nc.vector.reciprocal(out=rz194[:, 0:1],
                                 in_=zpairs.pop(194)[:, 0:1])
            rzpairs[194] = rz194
            emit_norm(194, w=1)
            nc.sync.dma_start(out=out_v[:, 184:, :],
                              in_=mean_all[:, 184:, :])

    nc.compile()
    return nc


def _get_nc():
    if "nc" not in _CACHE:
        _CACHE["nc"] = _build()
    return _CACHE["nc"]


def kernel(source_distance, context, kernel, bias, _trace=False, _tmpdir=None):
    from concourse.bass_utils import run_bass_kernel_spmd

    nc = _get_nc()

    source_distance = np.ascontiguousarray(source_distance, dtype=np.float32)
    context = np.ascontiguousarray(context, dtype=np.float32)
    kernel = np.ascontiguousarray(kernel, dtype=np.float32)
    bias = np.ascontiguousarray(bias, dtype=np.float32)

    in_maps = []
    for i in range(N_CORES):
        lo, hi = i * B_LOCAL, (i + 1) * B_LOCAL
        in_maps.append({
            "source_distance": source_distance[lo:hi],
            "context": context[lo:hi],
            "kernel": kernel,
            "bias": bias,
        })

    res = run_bass_kernel_spmd(nc, in_maps, list(range(N_CORES)),
                               trace=_trace, tmpdir=_tmpdir)
    out = np.concatenate([res.results[i]["out"] for i in range(N_CORES)], axis=0)
    if _trace:
        _CACHE["last_results"] = res
    return out


# revision 19
# speedup vs baseline: 1.1365x; 1.0801x over previous
"""Trainium2 Bass kernel for nn_Attention_42279658062045 (gnn_message_passing).

Computes, for each of B=200000 nodes:
    simi   = exp(-source_distance^2 / 2)                  [B, K]
    weight = softmax(simi @ kernel + bias, axis=-1)       [B, K]
    mean   = einsum('bk,bkd->bd', weight, context)        [B, D]

Sharding: pure data parallel over the node axis B across 8 NeuronCores;
kernel/bias replicated; no cross-device communication.

Per-core dataflow (B_LOCAL = 25000 rows, tiles of 128 rows, f32 throughout).
Rows are mapped to (partition, tile) as row = p*195 + t ("(p n)" layout), so
the source_distance load and the output store are fully contiguous per
partition (large DMA descriptors); context chunk descriptors are
30.7 KB/partition.

Work split per 128-row tile (K=30 weighted slabs of D=64). The softmax 1/Z
is applied to the 30 WEIGHTS (not the 64 outputs), so the k-reduction writes
the final output directly:
  - PE: transpose of simi tiles; logits = simi @ kernel + bias via two
    accumulating matmuls (ones-row stationary adds the bias).
  - ACT: batched square+exp of the distances, PSUM->SBUF copy of simi^T,
    exp(logits) with accum_out giving Z, the weight normalization
    wn = ew * (1/Z), and the weighted product for ACT_SLABS k-slabs.
  - GPSIMD: the weighted product for GP_SLABS k-slabs.
  - DVE: 1/Z reciprocal, weighted product for DVE_SLABS, and the full-30
    k-reduction over the interleaved product.

Engine-contention notes (HW-measured): a fp32 DVE tensor_tensor with two
SBUF input streams rides the SBUF read port shared with GPSIMD and runs
2-4x slower under overlap, so the DVE product reads its broadcast weight
operand from PSUM instead (dedicated PSUM read port; ACT writes a second
copy of the normalized weights there). fp32 tensor_reduce and reciprocal
are single-stream (1x perf mode) and are immune. No DVE op here is a
2-port perf-mode op (fp32 single-src copy/scalar), which would take the
shared port pair as an exclusive lock and block GPSIMD entirely.

The product tensor uses an interleaved layout [d_hi(32), k(30), d_lo(2)]
(flat addr = d_hi*60 + k*2 + d_lo) so the k-reduction reads at 8-byte
stride (2 hits per 16-byte SBUF line = full rate).
"""

import numpy as np

N_CORES = 8
B, K, D = 200000, 30, 64
B_LOCAL = B // N_CORES  # 25000
P = 128
CT = 3          # tiles per context DMA chunk
IL = 2          # product interleave: [d_hi(32), k(30), d_lo(IL)]
DH = D // IL    # 32
DVE_SLABS = 6   # k in [0, DVE_SLABS) multiplied on DVE
GP_SLABS = 21   # k in [DVE_SLABS, GP_END) multiplied on GPSIMD
GP_END = DVE_SLABS + GP_SLABS
ACT_SLABS = K - GP_END  # remaining slabs multiplied on ACT

_CACHE = {}


def _build():
    import concourse.bacc as bacc
    import concourse.tile as tile
    from concourse import mybir
    from concourse.masks import make_identity

    fp32 = mybir.dt.float32
    AF = mybir.ActivationFunctionType
    nc = bacc.Bacc("TRN2", target_bir_lowering=False, debug=False,
                   num_devices=N_CORES)

    dist = nc.dram_tensor("source_distance", [B_LOCAL, K], fp32,
                          kind="ExternalInput").ap()
    ctx_d = nc.dram_tensor("context", [B_LOCAL, K, D], fp32,
                           kind="ExternalInput").ap()
    kern = nc.dram_tensor("kernel", [K, K], fp32, kind="ExternalInput").ap()
    bias = nc.dram_tensor("bias", [K], fp32, kind="ExternalInput").ap()
    out = nc.dram_tensor("out", [B_LOCAL, D], fp32, kind="ExternalOutput").ap()

    n_full = B_LOCAL // P          # 195 full tiles
    rem = B_LOCAL - n_full * P     # 40 leftover rows

    # row(p, t) = p*n_full + t: per-partition-contiguous HBM runs.
    dist_v = dist[:n_full * P, :].rearrange("(p n) k -> p n k", p=P)
    ctx_v = ctx_d[:n_full * P].rearrange("(p n) k d -> p n (k d)", p=P)
    out_v = out[:n_full * P, :].rearrange("(p n) d -> p n d", p=P)

    with tile.TileContext(nc) as tc:
        from contextlib import ExitStack
        with ExitStack() as st:
            consts = st.enter_context(tc.tile_pool(name="consts", bufs=1))
            big = st.enter_context(tc.tile_pool(name="big", bufs=1))
            ctxp = st.enter_context(tc.tile_pool(name="ctx", bufs=3))
            prodp = st.enter_context(tc.tile_pool(name="prod", bufs=4))
            small = st.enter_context(tc.tile_pool(name="small", bufs=5))
            remp = st.enter_context(tc.tile_pool(name="rem", bufs=1))
            rzp = st.enter_context(tc.tile_pool(name="rz", bufs=10))
            psum_t = st.enter_context(
                tc.tile_pool(name="psumT", bufs=2, space="PSUM"))
            psum_l = st.enter_context(
                tc.tile_pool(name="psumL", bufs=2, space="PSUM"))
            psum_w = st.enter_context(
                tc.tile_pool(name="psumW", bufs=4, space="PSUM"))

            ident = consts.tile([P, P], fp32)
            make_identity(nc, ident)
            kern_s = consts.tile([K, K], fp32)
            nc.sync.dma_start(out=kern_s, in_=kern)
            bias_s = consts.tile([1, K], fp32)
            nc.sync.dma_start(out=bias_s, in_=bias.unsqueeze(0))
            ones_s = consts.tile([1, P], fp32)
            nc.vector.memset(ones_s, 1.0)

            # All distances for the full tiles; squared+exp'd in place.
            # The DMA comes in 4 big quarters (few, large transfers), but
            # the square/exp runs in 12-tile ACT slices (a monolithic
            # square/exp would monopolize ACT for ~3us and bubble the
            # pipeline).
            simi_all = big.tile([P, n_full, K], fp32)
            SL = 12
            simi_q = (n_full + 3) // 4

            def load_dist_quarter(r0):
                r1 = min(r0 + simi_q, n_full)
                nc.sync.dma_start(out=simi_all[:, r0:r1, :],
                                  in_=dist_v[:, r0:r1, :])

            def simi_prep_slice(r0):
                r1 = min(r0 + SL, n_full)
                nc.scalar.activation(out=simi_all[:, r0:r1, :],
                                     in_=simi_all[:, r0:r1, :], func=AF.Square)
                nc.scalar.activation(out=simi_all[:, r0:r1, :],
                                     in_=simi_all[:, r0:r1, :], func=AF.Exp,
                                     scale=-0.5)

            # Staged output for the full tiles.
            mean_all = big.tile([P, n_full, D], fp32)

            def logits_exp(simi_ap, expw_ap, zs_ap, rows):
                """simi [rows, K] -> expw = exp(simi @ kernel + bias),
                zs = sum(expw) per row."""
                simiT_p = psum_t.tile([K, P], fp32, tag="simiT_p")
                nc.tensor.transpose(out=simiT_p[:, :rows], in_=simi_ap,
                                    identity=ident[:rows, :rows])
                simiT_s = small.tile([K, P], fp32, tag="simiT_s")
                nc.scalar.copy(out=simiT_s[:, :rows], in_=simiT_p[:, :rows])

                logits_p = psum_l.tile([P, K], fp32, tag="logits_p")
                nc.tensor.matmul(out=logits_p[:rows, :],
                                 lhsT=simiT_s[:, :rows], rhs=kern_s,
                                 start=True, stop=False)
                nc.tensor.matmul(out=logits_p[:rows, :],
                                 lhsT=ones_s[:, :rows], rhs=bias_s,
                                 start=False, stop=True)

                nc.scalar.activation(out=expw_ap, in_=logits_p[:rows, :],
                                     func=AF.Exp, accum_out=zs_ap)

            # Remainder rows (partial tile): emitted first so its serial
            # chain overlaps the main pipeline.
            if rem:
                simi_r = remp.tile([P, K], fp32, tag="simi_r")
                nc.sync.dma_start(out=simi_r[:rem, :], in_=dist[n_full * P:, :])
                nc.scalar.activation(out=simi_r[:rem, :], in_=simi_r[:rem, :],
                                     func=AF.Square)
                nc.scalar.activation(out=simi_r[:rem, :], in_=simi_r[:rem, :],
                                     func=AF.Exp, scale=-0.5)
                ctx_r = ctxp.tile([P, K * D], fp32, tag="ctx")
                nc.sync.dma_start(
                    out=ctx_r[:rem, :],
                    in_=ctx_d[n_full * P:].rearrange("b k d -> b (k d)"))
                expw_r = remp.tile([P, K], fp32, tag="expw_r")
                zs_r = rzp.tile([P, 1], fp32, tag="zs_r")
                logits_exp(simi_r[:rem, :], expw_r[:rem, :], zs_r[:rem, :], rem)
                rz_r = rzp.tile([P, 1], fp32, tag="rz_r")
                nc.vector.reciprocal(out=rz_r[:rem, :], in_=zs_r[:rem, :])
                wn_r = remp.tile([P, K], fp32, tag="wn_r")
                nc.scalar.mul(out=wn_r[:rem, :], in_=expw_r[:rem, :],
                              mul=rz_r[:rem, :])
                pp_r = prodp.tile([P, 2, DH, K, IL], fp32, tag="prod")
                prod_r = pp_r[:, 0]
                ctx3r = ctx_r[:rem, :].rearrange("p (k d) -> p k d", k=K)
                nc.vector.tensor_mul(
                    out=prod_r.rearrange("p h k l -> p k h l")[:rem],
                    in0=ctx3r,
                    in1=wn_r[:rem, :].unsqueeze(2).broadcast_to([rem, K, D]))
                mean_r = remp.tile([P, D], fp32, tag="mean_r")
                nc.vector.reduce_sum(
                    out=mean_r[:rem, :].rearrange("p (h l) -> p h l", l=IL),
                    in_=prod_r.rearrange("p h k l -> p h l k")[:rem],
                    axis=mybir.AxisListType.X)
                nc.sync.dma_start(out=out[n_full * P:, :], in_=mean_r[:rem, :])

            # Full tiles: context DMA in CT-tile chunks; per-tile products
            # (interleaved) split DVE/GPSIMD/ACT, full-30 DVE reduce writing
            # the output tile directly.
            #
            # Software pipelining: the logits/exp chain (phase A) is emitted
            # one tile ahead of the product/reduce work (phase B). Products
            # use UNNORMALIZED exp weights, so they depend only on the exp;
            # the 1/Z is applied to the reduced pair outputs by GPSIMD, one
            # pair late, and the pair reduce itself is emitted one pair
            # late, so each pair's product ops precede the previous pair's
            # 4.1us reduce in the DVE queue and GPSIMD never starves.
            ews = {}
            zpairs = {}
            rzpairs = {}
            prodpairs = {}

            def phase_a(t):
                if t == 0:
                    # split so tile 0's simi is ready immediately
                    nc.sync.dma_start(out=simi_all[:, 0:SL, :],
                                      in_=dist_v[:, 0:SL, :])
                    nc.sync.dma_start(out=simi_all[:, SL:simi_q, :],
                                      in_=dist_v[:, SL:simi_q, :])
                    simi_prep_slice(0)
                elif t in (37, 86, 135):
                    load_dist_quarter({37: 1, 86: 2, 135: 3}[t] * simi_q)
                if t > 0 and (t + 4) % SL == 0 and t + 4 < n_full:
                    simi_prep_slice(t + 4)
                ew = small.tile([P, K], fp32, tag="ew")
                if t % 2 == 0:
                    zpair = rzp.tile([P, 2], fp32, tag="zpair")
                    zpairs[t] = zpair
                zp = zpairs[t - t % 2]
                logits_exp(simi_all[:, t, :], ew, zp[:, t % 2:t % 2 + 1], P)
                ews[t] = ew
                if t % 2 == 1:
                    # one tiny immune DVE op per pair; consumed much later
                    # by the GPSIMD output normalization.
                    rzpair = rzp.tile([P, 2], fp32, tag="rzpair")
                    nc.vector.reciprocal(out=rzpair, in_=zp)
                    rzpairs[t - 1] = rzpair
                    del zpairs[t - 1]

            def emit_reduce(j0):
                """Reduce pair (j0, j0+1) -> mean_all; frees the prodpair."""
                pp = prodpairs.pop(j0)
                nc.vector.reduce_sum(
                    out=mean_all[:, j0:j0 + 2, :].rearrange(
                        "p j (h l) -> p j h l", l=IL),
                    in_=pp.rearrange("p j h k l -> p j h l k"),
                    axis=mybir.AxisListType.X)

            def emit_norm(j0, w=2):
                """ACT: mean_all pair (j0, j0+1) *= 1/Z (in place)."""
                rzp_t = rzpairs.pop(j0)
                for j in range(w):
                    nc.scalar.mul(out=mean_all[:, j0 + j, :],
                                  in_=mean_all[:, j0 + j, :],
                                  mul=rzp_t[:, j:j + 1])

            ctx_tile = None
            phase_a(0)
            for t in range(n_full):
                cc, lane = divmod(t, CT)
                if lane == 0:
                    ctx_tile = ctxp.tile([P, CT, K * D], fp32, tag="ctx")
                    lo = cc * CT
                    cn = min(CT, n_full - lo)
                    if cc == 0:
                        # split chunk 0 so tile 0's products start early
                        nc.sync.dma_start(out=ctx_tile[:, :1, :],
                                          in_=ctx_v[:, :1, :])
                        nc.sync.dma_start(out=ctx_tile[:, 1:cn, :],
                                          in_=ctx_v[:, 1:cn, :])
                    else:
                        nc.sync.dma_start(out=ctx_tile[:, :cn, :],
                                          in_=ctx_v[:, lo:lo + cn, :])
                ctx3 = ctx_tile[:, lane, :].rearrange("p (k d) -> p k d", k=K)

                if t + 1 < n_full:
                    phase_a(t + 1)

                ew = ews.pop(t)
                # PSUM copy of the DVE slabs' weights (dedicated DVE read
                # port; avoids the GPSIMD-shared SBUF port).
                ew_p = psum_w.tile([P, DVE_SLABS], fp32, tag="ew_p")
                nc.scalar.copy(out=ew_p, in_=ew[:, :DVE_SLABS])

                if t % 2 == 0:
                    prodpair_new = prodp.tile([P, 2, DH, K, IL], fp32,
                                              tag="prod")
                    prodpairs[t] = prodpair_new
                prodpair = prodpairs[t - t % 2]
                prod = prodpair[:, t % 2]
                prod_k = prod.rearrange("p h k l -> p k h l")
                # GPSIMD: product for slabs [DVE_SLABS, GP_END)
                nc.gpsimd.tensor_mul(
                    out=prod_k[:, DVE_SLABS:GP_END],
                    in0=ctx3[:, DVE_SLABS:GP_END, :],
                    in1=ew[:, DVE_SLABS:GP_END].unsqueeze(2).broadcast_to(
                        [P, GP_SLABS, D]))
                # DVE: product for slabs [0, DVE_SLABS), interleaved out
                nc.vector.tensor_mul(
                    out=prod_k[:, :DVE_SLABS],
                    in0=ctx3[:, :DVE_SLABS, :],
                    in1=ew_p.unsqueeze(2).broadcast_to([P, DVE_SLABS, D]))
                # ACT: product for the last ACT_SLABS slabs
                for k in range(GP_END, K):
                    nc.scalar.mul(out=prod[:, :, k, :],
                                  in_=ctx3[:, k, :], mul=ew[:, k:k + 1])

                # DVE: full-30 reduce at 8-byte stride over the PREVIOUS
                # pair (one-pair lag keeps this pair's product ops ahead
                # of it in the DVE queue).
                if t % 2 == 1 and t >= 3:
                    emit_reduce(t - 3)
                # ACT: 1/Z normalization of the pair reduced at t-3
                # (two-pair lag: the reduce is certainly done, so the norm
                # never head-of-line-blocks the ACT queue).
                if t % 2 == 0 and t >= 6:
                    emit_norm(t - 6)
                # stream the staged output in 8-tile slices of normalized
                # tiles (highest normalized tile after emit_norm: t-5).
                if t % 2 == 0 and t >= 6 and (t - 4) % 8 == 0:
                    nc.sync.dma_start(out=out_v[:, t - 12:t - 4, :],
                                      in_=mean_all[:, t - 12:t - 4, :])

            # drain: tiles 192, 193 (pair) and 194 (solo)
            emit_reduce(192)
            pp_last = prodpairs.pop(194)
            nc.vector.reduce_sum(
                out=mean_all[:, 194, :].rearrange("p (h l) -> p h l", l=IL),
                in_=pp_last[:, 0].rearrange("p h k l -> p h l k"),
                axis=mybir.AxisListType.X)
            emit_norm(190)
            emit_norm(192)
            nc.sync.dma_start(out=out_v[:, 184:192, :],
                              in_=mean_all[:, 184:192, :])
            rz194 = rzp.tile([P, 2], fp32, tag="rzpair")
            nc.vector.reciprocal(out=rz194[:, 0:1],
                                 in_=zpairs.pop(194)[:, 0:1])
            rzpairs[194] = rz194
            emit_norm(194, w=1)
            nc.sync.dma_start(out=out_v[:, 192:, :],
                              in_=mean_all[:, 192:, :])

    nc.compile()
    return nc


def _get_nc():
    if "nc" not in _CACHE:
        _CACHE["nc"] = _build()
    return _CACHE["nc"]


def kernel(source_distance, context, kernel, bias, _trace=False, _tmpdir=None):
    from concourse.bass_utils import run_bass_kernel_spmd

    nc = _get_nc()

    source_distance = np.ascontiguousarray(source_distance, dtype=np.float32)
    context = np.ascontiguousarray(context, dtype=np.float32)
    kernel = np.ascontiguousarray(kernel, dtype=np.float32)
    bias = np.ascontiguousarray(bias, dtype=np.float32)

    in_maps = []
    for i in range(N_CORES):
        lo, hi = i * B_LOCAL, (i + 1) * B_LOCAL
        in_maps.append({
            "source_distance": source_distance[lo:hi],
            "context": context[lo:hi],
            "kernel": kernel,
            "bias": bias,
        })

    res = run_bass_kernel_spmd(nc, in_maps, list(range(N_CORES)),
                               trace=_trace, tmpdir=_tmpdir)
    out = np.concatenate([res.results[i]["out"] for i in range(N_CORES)], axis=0)
    if _trace:
        _CACHE["last_results"] = res
    return out


# revision 20
# speedup vs baseline: 1.1976x; 1.0537x over previous
"""Trainium2 Bass kernel for nn_Attention_42279658062045 (gnn_message_passing).

Computes, for each of B=200000 nodes:
    simi   = exp(-source_distance^2 / 2)                  [B, K]
    weight = softmax(simi @ kernel + bias, axis=-1)       [B, K]
    mean   = einsum('bk,bkd->bd', weight, context)        [B, D]

Sharding: pure data parallel over the node axis B across 8 NeuronCores;
kernel/bias replicated; no cross-device communication.

Per-core dataflow (B_LOCAL = 25000 rows, tiles of 128 rows, f32 throughout).
Rows are mapped to (partition, tile) as row = p*195 + t ("(p n)" layout), so
the source_distance load and the output store are fully contiguous per
partition (large DMA descriptors); context chunk descriptors are
30.7 KB/partition.

Work split per 128-row tile (K=30 weighted slabs of D=64). The softmax 1/Z
is applied to the 30 WEIGHTS (not the 64 outputs), so the k-reduction writes
the final output directly:
  - PE: transpose of simi tiles; logits = simi @ kernel + bias via two
    accumulating matmuls (ones-row stationary adds the bias).
  - ACT: batched square+exp of the distances, PSUM->SBUF copy of simi^T,
    exp(logits) with accum_out giving Z, the weight normalization
    wn = ew * (1/Z), and the weighted product for ACT_SLABS k-slabs.
  - GPSIMD: the weighted product for GP_SLABS k-slabs.
  - DVE: 1/Z reciprocal, weighted product for DVE_SLABS, and the full-30
    k-reduction over the interleaved product.

Engine-contention notes (HW-measured): a fp32 DVE tensor_tensor with two
SBUF input streams rides the SBUF read port shared with GPSIMD and runs
2-4x slower under overlap, so the DVE product reads its broadcast weight
operand from PSUM instead (dedicated PSUM read port; ACT writes a second
copy of the normalized weights there). fp32 tensor_reduce and reciprocal
are single-stream (1x perf mode) and are immune. No DVE op here is a
2-port perf-mode op (fp32 single-src copy/scalar), which would take the
shared port pair as an exclusive lock and block GPSIMD entirely.

The product tensor uses an interleaved layout [d_hi(32), k(30), d_lo(2)]
(flat addr = d_hi*60 + k*2 + d_lo) so the k-reduction reads at 8-byte
stride (2 hits per 16-byte SBUF line = full rate).
"""

import numpy as np

N_CORES = 8
B, K, D = 200000, 30, 64
B_LOCAL = B // N_CORES  # 25000
P = 128
CT = 3          # tiles per context DMA chunk
IL = 2          # product interleave: [d_hi(32), k(30), d_lo(IL)]
DH = D // IL    # 32
DVE_SLABS = 6   # k in [0, DVE_SLABS) multiplied on DVE
GP_SLABS = 21   # k in [DVE_SLABS, GP_END) multiplied on GPSIMD
GP_END = DVE_SLABS + GP_SLABS
ACT_SLABS = K - GP_END  # remaining slabs multiplied on ACT

_CACHE = {}


def _build():
    import concourse.bacc as bacc
    import concourse.tile as tile
    from concourse import mybir
    from concourse.masks import make_identity

    fp32 = mybir.dt.float32
    AF = mybir.ActivationFunctionType
    nc = bacc.Bacc("TRN2", target_bir_lowering=False, debug=False,
                   num_devices=N_CORES)

    dist = nc.dram_tensor("source_distance", [B_LOCAL, K], fp32,
                          kind="ExternalInput").ap()
    ctx_d = nc.dram_tensor("context", [B_LOCAL, K, D], fp32,
                           kind="ExternalInput").ap()
    kern = nc.dram_tensor("kernel", [K, K], fp32, kind="ExternalInput").ap()
    bias = nc.dram_tensor("bias", [K], fp32, kind="ExternalInput").ap()
    out = nc.dram_tensor("out", [B_LOCAL, D], fp32, kind="ExternalOutput").ap()

    n_full = B_LOCAL // P          # 195 full tiles
    rem = B_LOCAL - n_full * P     # 40 leftover rows

    # row(p, t) = p*n_full + t: per-partition-contiguous HBM runs.
    dist_v = dist[:n_full * P, :].rearrange("(p n) k -> p n k", p=P)
    ctx_v = ctx_d[:n_full * P].rearrange("(p n) k d -> p n (k d)", p=P)
    out_v = out[:n_full * P, :].rearrange("(p n) d -> p n d", p=P)

    with tile.TileContext(nc) as tc:
        from contextlib import ExitStack
        with ExitStack() as st:
            consts = st.enter_context(tc.tile_pool(name="consts", bufs=1))
            big = st.enter_context(tc.tile_pool(name="big", bufs=1))
            ctxp = st.enter_context(tc.tile_pool(name="ctx", bufs=3))
            prodp = st.enter_context(tc.tile_pool(name="prod", bufs=4))
            small = st.enter_context(tc.tile_pool(name="small", bufs=5))
            remp = st.enter_context(tc.tile_pool(name="rem", bufs=1))
            rzp = st.enter_context(tc.tile_pool(name="rz", bufs=10))
            psum_t = st.enter_context(
                tc.tile_pool(name="psumT", bufs=2, space="PSUM"))
            psum_l = st.enter_context(
                tc.tile_pool(name="psumL", bufs=2, space="PSUM"))
            psum_w = st.enter_context(
                tc.tile_pool(name="psumW", bufs=4, space="PSUM"))

            ident = consts.tile([P, P], fp32)
            make_identity(nc, ident)
            kern_s = consts.tile([K, K], fp32)
            nc.sync.dma_start(out=kern_s, in_=kern)
            bias_s = consts.tile([1, K], fp32)
            nc.sync.dma_start(out=bias_s, in_=bias.unsqueeze(0))
            ones_s = consts.tile([1, P], fp32)
            nc.vector.memset(ones_s, 1.0)

            # All distances for the full tiles; squared+exp'd in place.
            # The DMA comes in 4 big quarters (few, large transfers), but
            # the square/exp runs in 12-tile ACT slices (a monolithic
            # square/exp would monopolize ACT for ~3us and bubble the
            # pipeline).
            simi_all = big.tile([P, n_full, K], fp32)
            SL = 12
            simi_q = (n_full + 3) // 4

            def load_dist_quarter(r0):
                r1 = min(r0 + simi_q, n_full)
                nc.scalar.dma_start(out=simi_all[:, r0:r1, :],
                                    in_=dist_v[:, r0:r1, :])

            def simi_prep_slice(r0):
                r1 = min(r0 + SL, n_full)
                nc.scalar.activation(out=simi_all[:, r0:r1, :],
                                     in_=simi_all[:, r0:r1, :], func=AF.Square)
                nc.scalar.activation(out=simi_all[:, r0:r1, :],
                                     in_=simi_all[:, r0:r1, :], func=AF.Exp,
                                     scale=-0.5)

            # Staged output for the full tiles.
            mean_all = big.tile([P, n_full, D], fp32)

            def logits_exp(simi_ap, expw_ap, zs_ap, rows):
                """simi [rows, K] -> expw = exp(simi @ kernel + bias),
                zs = sum(expw) per row."""
                simiT_p = psum_t.tile([K, P], fp32, tag="simiT_p")
                nc.tensor.transpose(out=simiT_p[:, :rows], in_=simi_ap,
                                    identity=ident[:rows, :rows])
                simiT_s = small.tile([K, P], fp32, tag="simiT_s")
                nc.scalar.copy(out=simiT_s[:, :rows], in_=simiT_p[:, :rows])

                logits_p = psum_l.tile([P, K], fp32, tag="logits_p")
                nc.tensor.matmul(out=logits_p[:rows, :],
                                 lhsT=simiT_s[:, :rows], rhs=kern_s,
                                 start=True, stop=False)
                nc.tensor.matmul(out=logits_p[:rows, :],
                                 lhsT=ones_s[:, :rows], rhs=bias_s,
                                 start=False, stop=True)

                nc.scalar.activation(out=expw_ap, in_=logits_p[:rows, :],
                                     func=AF.Exp, accum_out=zs_ap)

            # Remainder rows (partial tile): emitted first so its serial
            # chain overlaps the main pipeline.
            if rem:
                simi_r = remp.tile([P, K], fp32, tag="simi_r")
                nc.scalar.dma_start(out=simi_r[:rem, :],
                                    in_=dist[n_full * P:, :])
                nc.scalar.activation(out=simi_r[:rem, :], in_=simi_r[:rem, :],
                                     func=AF.Square)
                nc.scalar.activation(out=simi_r[:rem, :], in_=simi_r[:rem, :],
                                     func=AF.Exp, scale=-0.5)
                ctx_r = ctxp.tile([P, K * D], fp32, tag="ctx")
                nc.sync.dma_start(
                    out=ctx_r[:rem, :],
                    in_=ctx_d[n_full * P:].rearrange("b k d -> b (k d)"))
                expw_r = remp.tile([P, K], fp32, tag="expw_r")
                zs_r = rzp.tile([P, 1], fp32, tag="zs_r")
                logits_exp(simi_r[:rem, :], expw_r[:rem, :], zs_r[:rem, :], rem)
                rz_r = rzp.tile([P, 1], fp32, tag="rz_r")
                nc.vector.reciprocal(out=rz_r[:rem, :], in_=zs_r[:rem, :])
                wn_r = remp.tile([P, K], fp32, tag="wn_r")
                nc.scalar.mul(out=wn_r[:rem, :], in_=expw_r[:rem, :],
                              mul=rz_r[:rem, :])
                pp_r = prodp.tile([P, 2, DH, K, IL], fp32, tag="prod")
                prod_r = pp_r[:, 0]
                ctx3r = ctx_r[:rem, :].rearrange("p (k d) -> p k d", k=K)
                nc.vector.tensor_mul(
                    out=prod_r.rearrange("p h k l -> p k h l")[:rem],
                    in0=ctx3r,
                    in1=wn_r[:rem, :].unsqueeze(2).broadcast_to([rem, K, D]))
                mean_r = remp.tile([P, D], fp32, tag="mean_r")
                nc.vector.reduce_sum(
                    out=mean_r[:rem, :].rearrange("p (h l) -> p h l", l=IL),
                    in_=prod_r.rearrange("p h k l -> p h l k")[:rem],
                    axis=mybir.AxisListType.X)
                nc.scalar.dma_start(out=out[n_full * P:, :],
                                    in_=mean_r[:rem, :])

            # Full tiles: context DMA in CT-tile chunks; per-tile products
            # (interleaved) split DVE/GPSIMD/ACT, full-30 DVE reduce writing
            # the output tile directly.
            #
            # Software pipelining: the logits/exp chain (phase A) is emitted
            # one tile ahead of the product/reduce work (phase B). Products
            # use UNNORMALIZED exp weights, so they depend only on the exp;
            # the 1/Z is applied to the reduced pair outputs by GPSIMD, one
            # pair late, and the pair reduce itself is emitted one pair
            # late, so each pair's product ops precede the previous pair's
            # 4.1us reduce in the DVE queue and GPSIMD never starves.
            ews = {}
            zpairs = {}
            rzpairs = {}
            prodpairs = {}

            def phase_a(t):
                if t == 0:
                    # split so tile 0's simi is ready immediately
                    nc.scalar.dma_start(out=simi_all[:, 0:SL, :],
                                        in_=dist_v[:, 0:SL, :])
                    nc.scalar.dma_start(out=simi_all[:, SL:simi_q, :],
                                        in_=dist_v[:, SL:simi_q, :])
                    simi_prep_slice(0)
                elif t in (37, 86, 135):
                    load_dist_quarter({37: 1, 86: 2, 135: 3}[t] * simi_q)
                if t > 0 and (t + 4) % SL == 0 and t + 4 < n_full:
                    simi_prep_slice(t + 4)
                ew = small.tile([P, K], fp32, tag="ew")
                if t % 2 == 0:
                    zpair = rzp.tile([P, 2], fp32, tag="zpair")
                    zpairs[t] = zpair
                zp = zpairs[t - t % 2]
                logits_exp(simi_all[:, t, :], ew, zp[:, t % 2:t % 2 + 1], P)
                ews[t] = ew
                if t % 2 == 1:
                    # one tiny immune DVE op per pair; consumed much later
                    # by the GPSIMD output normalization.
                    rzpair = rzp.tile([P, 2], fp32, tag="rzpair")
                    nc.vector.reciprocal(out=rzpair, in_=zp)
                    rzpairs[t - 1] = rzpair
                    del zpairs[t - 1]

            def emit_reduce(j0):
                """Reduce pair (j0, j0+1) -> mean_all; frees the prodpair."""
                pp = prodpairs.pop(j0)
                nc.vector.reduce_sum(
                    out=mean_all[:, j0:j0 + 2, :].rearrange(
                        "p j (h l) -> p j h l", l=IL),
                    in_=pp.rearrange("p j h k l -> p j h l k"),
                    axis=mybir.AxisListType.X)

            def emit_norm(j0, w=2):
                """ACT: mean_all pair (j0, j0+1) *= 1/Z (in place)."""
                rzp_t = rzpairs.pop(j0)
                for j in range(w):
                    nc.scalar.mul(out=mean_all[:, j0 + j, :],
                                  in_=mean_all[:, j0 + j, :],
                                  mul=rzp_t[:, j:j + 1])

            ctx_tile = None
            phase_a(0)
            for t in range(n_full):
                cc, lane = divmod(t, CT)
                if lane == 0:
                    ctx_tile = ctxp.tile([P, CT, K * D], fp32, tag="ctx")
                    lo = cc * CT
                    cn = min(CT, n_full - lo)
                    if cc == 0:
                        # split chunk 0 so tile 0's products start early
                        nc.sync.dma_start(out=ctx_tile[:, :1, :],
                                          in_=ctx_v[:, :1, :])
                        nc.sync.dma_start(out=ctx_tile[:, 1:cn, :],
                                          in_=ctx_v[:, 1:cn, :])
                    else:
                        nc.sync.dma_start(out=ctx_tile[:, :cn, :],
                                          in_=ctx_v[:, lo:lo + cn, :])
                ctx3 = ctx_tile[:, lane, :].rearrange("p (k d) -> p k d", k=K)

                if t + 1 < n_full:
                    phase_a(t + 1)

                ew = ews.pop(t)
                # PSUM copy of the DVE slabs' weights (dedicated DVE read
                # port; avoids the GPSIMD-shared SBUF port).
                ew_p = psum_w.tile([P, DVE_SLABS], fp32, tag="ew_p")
                nc.scalar.copy(out=ew_p, in_=ew[:, :DVE_SLABS])

                if t % 2 == 0:
                    prodpair_new = prodp.tile([P, 2, DH, K, IL], fp32,
                                              tag="prod")
                    prodpairs[t] = prodpair_new
                prodpair = prodpairs[t - t % 2]
                prod = prodpair[:, t % 2]
                prod_k = prod.rearrange("p h k l -> p k h l")
                # GPSIMD: product for slabs [DVE_SLABS, GP_END)
                nc.gpsimd.tensor_mul(
                    out=prod_k[:, DVE_SLABS:GP_END],
                    in0=ctx3[:, DVE_SLABS:GP_END, :],
                    in1=ew[:, DVE_SLABS:GP_END].unsqueeze(2).broadcast_to(
                        [P, GP_SLABS, D]))
                # DVE: product for slabs [0, DVE_SLABS), interleaved out
                nc.vector.tensor_mul(
                    out=prod_k[:, :DVE_SLABS],
                    in0=ctx3[:, :DVE_SLABS, :],
                    in1=ew_p.unsqueeze(2).broadcast_to([P, DVE_SLABS, D]))
                # ACT: product for the last ACT_SLABS slabs
                for k in range(GP_END, K):
                    nc.scalar.mul(out=prod[:, :, k, :],
                                  in_=ctx3[:, k, :], mul=ew[:, k:k + 1])

                # DVE: full-30 reduce at 8-byte stride over the PREVIOUS
                # pair (one-pair lag keeps this pair's product ops ahead
                # of it in the DVE queue).
                if t % 2 == 1 and t >= 3:
                    emit_reduce(t - 3)
                # ACT: 1/Z normalization of the pair reduced at t-3
                # (two-pair lag: the reduce is certainly done, so the norm
                # never head-of-line-blocks the ACT queue).
                if t % 2 == 0 and t >= 6:
                    emit_norm(t - 6)
                # stream the staged output in 8-tile slices of normalized
                # tiles (highest normalized tile after emit_norm: t-5).
                if t % 2 == 0 and t >= 6 and (t - 4) % 8 == 0:
                    nc.scalar.dma_start(out=out_v[:, t - 12:t - 4, :],
                                        in_=mean_all[:, t - 12:t - 4, :])

            # drain: tiles 192, 193 (pair) and 194 (solo)
            emit_reduce(192)
            pp_last = prodpairs.pop(194)
            nc.vector.reduce_sum(
                out=mean_all[:, 194, :].rearrange("p (h l) -> p h l", l=IL),
                in_=pp_last[:, 0].rearrange("p h k l -> p h l k"),
                axis=mybir.AxisListType.X)
            emit_norm(190)
            emit_norm(192)
            nc.scalar.dma_start(out=out_v[:, 184:192, :],
                                in_=mean_all[:, 184:192, :])
            rz194 = rzp.tile([P, 2], fp32, tag="rzpair")
            nc.vector.reciprocal(out=rz194[:, 0:1],
                                 in_=zpairs.pop(194)[:, 0:1])
            rzpairs[194] = rz194
            emit_norm(194, w=1)
            nc.scalar.dma_start(out=out_v[:, 192:, :],
                                in_=mean_all[:, 192:, :])

    nc.compile()
    return nc


def _get_nc():
    if "nc" not in _CACHE:
        _CACHE["nc"] = _build()
    return _CACHE["nc"]


def kernel(source_distance, context, kernel, bias, _trace=False, _tmpdir=None):
    from concourse.bass_utils import run_bass_kernel_spmd

    nc = _get_nc()

    source_distance = np.ascontiguousarray(source_distance, dtype=np.float32)
    context = np.ascontiguousarray(context, dtype=np.float32)
    kernel = np.ascontiguousarray(kernel, dtype=np.float32)
    bias = np.ascontiguousarray(bias, dtype=np.float32)

    in_maps = []
    for i in range(N_CORES):
        lo, hi = i * B_LOCAL, (i + 1) * B_LOCAL
        in_maps.append({
            "source_distance": source_distance[lo:hi],
            "context": context[lo:hi],
            "kernel": kernel,
            "bias": bias,
        })

    res = run_bass_kernel_spmd(nc, in_maps, list(range(N_CORES)),
                               trace=_trace, tmpdir=_tmpdir)
    out = np.concatenate([res.results[i]["out"] for i in range(N_CORES)], axis=0)
    if _trace:
        _CACHE["last_results"] = res
    return out
